# revision 16
# baseline (speedup 1.0000x reference)
"""Distance-loss kernel for Trainium2 (8 NeuronCores, data-parallel over batch).

loss = mean over (b, c != label_b) of sqrt(||x_b - center_c||^2)

Host-side staging is layout/dtype only (fp8 casts, transposed/tiled views,
and the centers[labels] row gather -- pure indexing); all arithmetic
(norms, matmuls, squares, sqrt, reductions) runs on device with fp32
accumulation.  x-hat = e4m3(x) and c-hat = e4m3(centers) are used
CONSISTENTLY everywhere, so d^2 = ||x-hat - c-hat||^2 is exact up to the
(averaged-out) quantization of the points themselves.

Per-core plan (B_shard = 2048 rows, distmat as psum[c, b]):
  - psum[c, b] = c_c . x_b via fp8 DoubleRow matmuls: K=256 in ONE pass
    per (c-tile, 512-col b-window) -- 32 passes instead of 64.
  - -0.5||x_b||^2 rides each window as a K=1 aug matmul; the four augs of
    a c-tile run CONCURRENTLY via row tiling (tile_position=(32j, 0)).
    Their rhs rows sit at partitions 0/32/64/96 of xx4, produced by a
    col-tiled ones-matmul reduce (tile_position=(0, 32j)) of the x-hat
    squares, computed in parallel on DVE (k-half 0) and a ScalarE Square
    activation (k-half 1).
  - ScalarE chain: d = sqrt(-2 psum + ||c_c||^2) with the class norm as
    per-partition bias, accumulating sum_b d into acc[:, m].  The chain
    runs at its (N+352)/1.2ns floor; everything else hides under it.
  - label-entry correction: centers[labels] rows are staged host-side
    (indexing only); sum_d (x-g)^2 entirely on DVE (GPSIMD tensor ops
    starve DVE via SBUF port contention -- keep GPSIMD to DMA only),
    3D tensor_reduce, one small sqrt into acc[:, 8:24] (no accumulator).
  - scheduling pins: long DVE ops that become ready early are WAW-guarded
    behind the chain-gating xx copy via tiny bridge copies, otherwise the
    Tile list scheduler hoists them and stalls the ScalarE chain; the DMAs
    are one-per-queue for the critical tensors because a queue's 2nd DMA
    completes ~2.3us after its 1st; an 13-rep PE warmup holds the clock
    governor up until the first real matmuls.
  - out = acc [128, 24] fp32 per core; host sums cols 0..7 minus cols
    8..23 across cores and divides by B*(C-1).
"""

import sys
from contextlib import ExitStack

import numpy as np

if "/opt/trn_rl_repo" not in sys.path:
    sys.path.insert(0, "/opt/trn_rl_repo")

import ml_dtypes

import concourse.bass as bass
import concourse.mybir as mybir
from concourse.bacc import Bacc
from concourse.tile import TileContext

F32 = mybir.dt.float32
BF16 = mybir.dt.bfloat16
FP8 = mybir.dt.float8e4
AF = mybir.ActivationFunctionType
ALU = mybir.AluOpType
PM = mybir.MatmulPerfMode
BF = ml_dtypes.bfloat16
F8 = ml_dtypes.float8_e4m3

N_CORES = 8
B = 16384
C = 1000
D = 256
BS = B // N_CORES          # 2048 rows per core
T = BS // 128              # 16 row tiles per core
NC_TILES = 8               # class tiles (last one has 104 classes)


def build_nc() -> bass.Bass:
    nc = Bacc()
    # xT8[p, o, b] = x8[b, o*128+p]   (o-major k halves, fp8)
    # cT8[p, o, c] = c8[c, o*128+p]
    # cp[p, m*256:(m+1)*256] = c8[m*128+p, :]        (pad rows zero)
    # xp[p, t*256:(t+1)*256] = x8[t*128+p, :]
    # xg[p, t*256:(t+1)*256] = c8[labels[t*128+p], :]
    xT8_d = nc.dram_tensor("xT8", [128, 2, BS], FP8, kind="ExternalInput")
    cb_d = nc.dram_tensor("cb", [128, 2 * 1024 + NC_TILES * D], FP8,
                          kind="ExternalInput")
    xb_d = nc.dram_tensor("xb", [128, 2 * T * D], FP8, kind="ExternalInput")
    o_d = nc.dram_tensor("out", [128, 8 + T], F32, kind="ExternalOutput")

    with TileContext(nc) as tc, ExitStack() as ctx:
        const = ctx.enter_context(tc.tile_pool(name="const", bufs=1))
        dpool = ctx.enter_context(tc.tile_pool(name="dpool", bufs=2))
        mmps = ctx.enter_context(tc.tile_pool(name="mmps", bufs=2, space="PSUM"))

        # ---- input DMAs ----
        # sync: xT8 halves then xp; scalar: cT8, cp, xg.
        xT8 = const.tile([128, 2, BS], FP8)
        nc.sync.dma_start(out=xT8[:, 0, :], in_=xT8_d[:, 0, :])
        nc.scalar.dma_start(out=xT8[:, 1, :], in_=xT8_d[:, 1, :])
        cb = const.tile([128, 2 * 1024 + NC_TILES * D], FP8)
        nc.gpsimd.dma_start(out=cb[:], in_=cb_d[:, :])
        cT8 = cb[:, 0:2048].rearrange("p (o c) -> p o c", o=2)
        cp = cb[:, 2048:]
        xb = const.tile([128, 2 * T * D], FP8)
        nc.gpsimd.dma_start(out=xb[:], in_=xb_d[:, :])
        xp = xb[:, 0:T * D]
        xg = xb[:, T * D:]

        # ---- constants (DVE queue head) ----
        ones128 = const.tile([128, 128], BF16)
        nc.vector.memset(ones128[:], 1.0)
        acc = const.tile([128, 8 + T], F32)
        nc.vector.memset(acc[:], 0.0)
        wu_w = const.tile([128, 4], BF16)
        nc.vector.memset(wu_w[:], 0.5)
        wu_r = const.tile([128, 512], BF16)
        nc.vector.memset(wu_r[:], 0.25)
        dum0 = const.tile([128, 1], F32)
        nc.vector.memset(dum0[:], 1.0)

        # ---- ScalarE head: square table, k-half-1 squares, cc0, sqrt table
        dumsq = const.tile([128, 1], BF16)
        nc.scalar.activation(dumsq[:], dum0[:], AF.Square)
        sqs = const.tile([128, 2, BS], BF16)
        nc.scalar.activation(sqs[:, 1, :], xT8[:, 1, :], AF.Square)
        dum1 = const.tile([128, 1], F32)
        nc.scalar.activation(dum1[:], dum0[:], AF.Sqrt)
        cc = const.tile([128, NC_TILES], F32)

        # ---- DVE: k-half-0 squares, then xx copy + remaining cc ----
        nc.vector.tensor_tensor(out=sqs[:, 0, :], in0=xT8[:, 0, :],
                                in1=xT8[:, 0, :], op=ALU.mult)

        # ---- PE: warmup, DR mains m0, ones-reduce, augs, mains m1.. ----
        pswu = mmps.tile([128, 2048], F32, tag="mm")
        for rep in range(13):
            nc.tensor.matmul(pswu[0:4, 0:512], wu_w[:], wu_r[:],
                             start=(rep == 0), stop=(rep == 12))
        psxx = mmps.tile([128, 2048], F32, tag="mm")

        ps_tiles = [mmps.tile([128, 2048], F32, tag="mm", name=f"ps{m}")
                    for m in range(NC_TILES)]
        dt_tiles = [dpool.tile([128, 2048], BF16, tag="dt", name=f"dt{m}")
                    for m in range(NC_TILES)]

        def dr_mains(m):
            cnt = min(128, C - m * 128)
            ps = ps_tiles[m]
            for j in range(4):
                nc.tensor.matmul(
                    ps[0:cnt, j * 512:(j + 1) * 512],
                    cT8[:, :, m * 128:m * 128 + cnt],
                    xT8[:, :, j * 512:(j + 1) * 512],
                    start=True, stop=False, perf_mode=PM.DoubleRow)

        def augs(m):
            cnt = min(128, C - m * 128)
            ps = ps_tiles[m]
            for j in range(4):
                nc.tensor.matmul(
                    ps[0:cnt, j * 512:(j + 1) * 512],
                    ones128[32 * j:32 * j + 1, 0:cnt],
                    xx4[32 * j:32 * j + 1, :],
                    start=False, stop=True,
                    tile_position=(32 * j, 0))

        # ones-reduce of squares into psxx rows 32j (col-tiled)
        for j in range(4):
            for o in range(2):
                nc.tensor.matmul(
                    psxx[32 * j:32 * j + 1, 0:512],
                    ones128[:, 32 * j:32 * j + 1],
                    sqs[:, o, j * 512:(j + 1) * 512],
                    start=(o == 0), stop=(o == 1),
                    tile_position=(0, 32 * j))

        dr_mains(0)

        # xx copy: full-width strided-free copy keeps rows at 32j
        xx4 = const.tile([128, 512], BF16)
        nc.vector.tensor_scalar(out=xx4[:], in0=psxx[:, 0:512], scalar1=-0.5,
                                scalar2=None, op0=ALU.mult)

        # class-norm biases on DVE: first two fill the pre-xx idle, the
        # rest are WAW-guarded behind the xx copy so they cannot delay it
        csqA = const.tile([128, D], BF16)
        for m in range(2):
            nc.vector.scalar_tensor_tensor(
                out=csqA[:], in0=cp[:, m * D:(m + 1) * D], scalar=0.0,
                in1=cp[:, m * D:(m + 1) * D], op0=ALU.bypass, op1=ALU.mult,
                accum_out=cc[:, m:m + 1])

        csqB = const.tile([128, D], BF16)
        nc.vector.tensor_copy(csqB[0:1, 0:2], xx4[0:1, 0:2])
        for m in range(2, NC_TILES):
            nc.vector.scalar_tensor_tensor(
                out=csqB[:], in0=cp[:, m * D:(m + 1) * D], scalar=0.0,
                in1=cp[:, m * D:(m + 1) * D], op0=ALU.bypass, op1=ALU.mult,
                accum_out=cc[:, m:m + 1])

        augs(0)
        dr_mains(1)
        augs(1)
        for m in range(2, NC_TILES):
            dr_mains(m)
            augs(m)

        # ---- ScalarE sqrt chain ----
        for m in range(NC_TILES):
            cnt = min(128, C - m * 128)
            nc.scalar.activation(
                dt_tiles[m][0:cnt, :], ps_tiles[m][0:cnt, :], AF.Sqrt,
                bias=cc[0:cnt, m:m + 1], scale=-2.0,
                accum_out=acc[0:cnt, m:m + 1])

        # ---- correction: sum_d (x8 - g8)^2 per row, split GPSIMD/DVE ----
        HB = T * D // 2  # 2048 cols per half
        df = const.tile([128, T * D], BF16)
        dsq = const.tile([128, T * D], BF16)
        dacc = const.tile([128, T], F32)
        d3a = dsq[:, 0:HB].rearrange("p (t d) -> p t d", d=D)
        d3b = dsq[:, HB:].rearrange("p (t d) -> p t d", d=D)
        # scheduler guard: both correction subs gain a WAW dep on this tiny
        # copy, which reads xx4 -- so neither can hoist ahead of the xx copy
        nc.vector.tensor_copy(df[0:1, HB - 1:HB + 1], xx4[0:1, 0:2])
        nc.vector.tensor_tensor(out=df[:, 0:HB], in0=xb[:, 0:HB],
                                in1=xg[:, 0:HB], op=ALU.subtract)
        nc.vector.tensor_tensor(out=dsq[:, 0:HB], in0=df[:, 0:HB],
                                in1=df[:, 0:HB], op=ALU.mult)
        nc.vector.reduce_sum(out=dacc[:, 0:T // 2], in_=d3a,
                             axis=mybir.AxisListType.X)
        nc.vector.tensor_tensor(out=df[:, HB:], in0=xb[:, HB:T * D],
                                in1=xg[:, HB:], op=ALU.subtract)
        nc.vector.tensor_tensor(out=dsq[:, HB:], in0=df[:, HB:],
                                in1=df[:, HB:], op=ALU.mult)
        nc.vector.reduce_sum(out=dacc[:, T // 2:], in_=d3b,
                             axis=mybir.AxisListType.X)

        nc.scalar.activation(acc[:, 8:], dacc[:], AF.Sqrt)

        nc.sync.dma_start(out=o_d[:, :], in_=acc[:])

    nc.compile()
    return nc


_NC_CACHE = None


def _get_nc():
    global _NC_CACHE
    if _NC_CACHE is None:
        _NC_CACHE = build_nc()
    return _NC_CACHE


def make_in_maps(x, centers, labels):
    x = np.asarray(x, dtype=np.float32)
    centers = np.asarray(centers, dtype=np.float32)
    labels = np.asarray(labels).astype(np.int64)
    x8 = x.astype(F8)
    c8 = centers.astype(F8)
    cT8 = np.zeros((128, 2, 1024), F8)
    cT8[:, :, :C] = c8.T.reshape(2, 128, C).transpose(1, 0, 2)
    cpad = np.zeros((NC_TILES * 128, D), F8)
    cpad[:C] = c8
    cp = np.ascontiguousarray(
        cpad.reshape(NC_TILES, 128, D).transpose(1, 0, 2).reshape(128, -1))
    cb = np.concatenate([cT8.reshape(128, -1), cp], axis=1)

    in_maps = []
    for i in range(N_CORES):
        xs8 = x8[i * BS:(i + 1) * BS]
        ls = labels[i * BS:(i + 1) * BS]
        xT8 = np.ascontiguousarray(
            xs8.T.reshape(2, 128, BS).transpose(1, 0, 2))
        xp = np.ascontiguousarray(
            xs8.reshape(T, 128, D).transpose(1, 0, 2).reshape(128, -1))
        g8 = c8[ls]  # host-side row gather (indexing only)
        xg = np.ascontiguousarray(
            g8.reshape(T, 128, D).transpose(1, 0, 2).reshape(128, -1))
        in_maps.append({
            "xT8": xT8, "cb": cb, "xb": np.concatenate([xp, xg], axis=1),
        })
    return in_maps


def _ensure_ntff_hook_module():
    """Provide antenv.axon_hooks if the image's antenv package lacks it.

    concourse.bass_utils imports it for trace=True under axon; the hook
    itself lives in libaxon_pjrt.so and is wrapped by trn_agent_boot.
    """
    import types

    try:
        import antenv.axon_hooks  # noqa: F401
        return
    except ImportError:
        pass
    mod = types.ModuleType("antenv.axon_hooks")
    state = {"hook": None}

    def set_axon_ntff_profile_hook(hook):
        state["hook"] = hook

    def get_axon_ntff_profile_hook():
        if state["hook"] is None:
            try:
                from trn_agent_boot.trn_boot import _ntff_profile_via_ctypes

                state["hook"] = _ntff_profile_via_ctypes(
                    "/opt/axon/libaxon_pjrt.so"
                )
            except Exception:
                return None
        return state["hook"]

    mod.set_axon_ntff_profile_hook = set_axon_ntff_profile_hook
    mod.get_axon_ntff_profile_hook = get_axon_ntff_profile_hook
    sys.modules["antenv.axon_hooks"] = mod
    try:
        import antenv

        antenv.axon_hooks = mod
    except ImportError:
        pass


def _run_once(nc, in_maps, _results_out, **run_kwargs):
    from concourse.bass_utils import run_bass_kernel_spmd

    res = run_bass_kernel_spmd(nc, in_maps, core_ids=list(range(N_CORES)),
                               **run_kwargs)
    if _results_out is not None:
        _results_out.append(res)
    total = 0.0
    for r in res.results:
        a = np.asarray(r["out"], dtype=np.float64)
        total += a[:, 0:8].sum() - a[:, 8:].sum()
    return total / (B * (C - 1))


def kernel(x, centers, labels, _results_out=None, **run_kwargs):
    _ensure_ntff_hook_module()
    nc = _get_nc()
    in_maps = make_in_maps(x, centers, labels)
    loss = _run_once(nc, in_maps, _results_out, **run_kwargs)
    if not np.isfinite(loss):
        # rare transient hardware flake right after NEFF load; retry once
        loss = _run_once(nc, in_maps, _results_out, **run_kwargs)
    return np.float32(loss)
